# revision 103
# baseline (speedup 1.0000x reference)
"""Trainium2 (8 NeuronCores, SPMD) kernel for windowed multi-head attention
with relative position bias (Swin-3D style block).

Strategy: pure data-parallel over the B=32 window axis — 4 windows per core,
no collectives. Per core:
  phase 1: qkv projection.  q produced TRANSPOSED (feature-on-partition) for
           the score matmuls; k produced transposed AND zero-padded to the
           full 128-partition contraction (head h's 64 features live in
           rows (h%2)*64..+64, the other 64 rows stay zero) so score
           matmuls keep the PE in its 128-row tile config — mixing 64-row
           and 128-row stationaries costs a ~100ns array-reconfig on every
           transition.  v produced natural (token-on-partition) with a
           column of ones appended (row 64 of the PV output then holds the
           softmax denominator for free).
  phase 2: per (head pair, window): both heads' scores^T land in one
           2-bank PSUM tile, one exp on ScalarE covers both, then multiply
           by exp(bias)^T (host-precomputed, fully SBUF-resident), PV
           matmuls accumulate attn_out^T [64+1, 512], normalize by the
           reciprocal of the sum row (partition-broadcast on the otherwise
           idle GpSimd engine).
  phase 3: output projection from attn_out^T tiles; result lands natural
           [token, feature] and is DMAed out.

All matmul operands are bf16 (full TensorE rate); accumulation fp32 in PSUM.
The softmax scale is folded into the q weights on the host. exp(s+b) is
computed as exp(s)*exp(b) — exact up to fp rounding, and lets the bias add
run as a cheap bf16 multiply on VectorE instead of an fp32 PSUM add.
v/proj biases enter the output linearly and are applied on the host (they
are zeros for this problem's inputs anyway).

Scheduling: the per-engine instruction streams are STATIC after Tile
scheduling.  qkv matmuls for the NEXT window and projection matmuls for the
PREVIOUS window are interleaved between score/PV instructions at
single-matmul granularity (generator-based fill queue) so the PE never
idles while ScalarE drains exp chains.  qkT/kpad/vsb/aoT are 2-slot
rotating buffers (window parity) so the full exp-bias table fits in SBUF;
x tiles are prefetched 2 windows ahead.
"""

import os

import numpy as np
import ml_dtypes

# the device can be left in a sticky slow mode (~+55us on every launch,
# persisting across runs) by prior sessions; a core reset at open clears it
os.environ.setdefault("NEURON_RT_RESET_CORES", "1")

B, NTOK, DIM = 32, 512, 768
NH, HD = 12, 64
NCORES = 8
BW = B // NCORES          # 4 windows per core
SCALE = HD ** -0.5
KT = NTOK // 128          # 4 token tiles
FT = DIM // 128           # 6 feature tiles

TRACE = False             # set by test.py to capture neuron-profile timing
LAST_RESULT = None        # BassKernelResults of the last run (for profiling)

_nc_cache = {}


def _build(has_bqk: bool):
    import concourse.mybir as mybir
    import concourse.tile as tile
    from concourse import bacc
    from contextlib import ExitStack

    dt = mybir.dt
    bf16, f32 = dt.bfloat16, dt.float32
    AF = mybir.ActivationFunctionType

    # Bacc (not plain Bass): its compile pass splits multi-semaphore waits
    # into EventSemaphore instructions — TRN2 allows only 1 wait per inst.
    nc = bacc.Bacc("TRN2", target_bir_lowering=False, debug=False)
    xT_d = nc.declare_dram_parameter("xT", [BW, DIM, NTOK], bf16, False)
    wq_d = nc.declare_dram_parameter("wqkvT", [DIM, 3 * DIM], bf16, False)
    wp_d = nc.declare_dram_parameter("wprojT", [DIM, DIM], bf16, False)
    eb_d = nc.declare_dram_parameter("expb", [128, NH, KT, NTOK], bf16, False)
    bq_d = nc.declare_dram_parameter("bqk", [128, 2 * FT], f32, False)
    out_d = nc.declare_dram_parameter("out", [BW, NTOK, DIM], bf16, True)

    ctx = ExitStack()
    with ctx:
        tc = ctx.enter_context(tile.TileContext(nc))
        const = ctx.enter_context(tc.tile_pool(name="const", bufs=1))
        xpool = ctx.enter_context(tc.tile_pool(name="xT", bufs=2))
        empool = ctx.enter_context(tc.tile_pool(name="expm", bufs=3))
        rpool = ctx.enter_context(tc.tile_pool(name="recip", bufs=3))
        opool = ctx.enter_context(tc.tile_pool(name="osb", bufs=2))
        ps_s = ctx.enter_context(tc.tile_pool(name="ps_s", bufs=2, space="PSUM"))
        ps_f = ctx.enter_context(tc.tile_pool(name="ps_f", bufs=2, space="PSUM"))
        ps_pv = ctx.enter_context(tc.tile_pool(name="ps_pv", bufs=1, space="PSUM"))

        xw_tiles = {}

        def prefetch_x(w):
            xw = xpool.tile([128, FT, NTOK], bf16, name="xw", tag="xw")
            # per-k chunks: the first qkv matmuls need only the k=0 slice
            for k in range(FT):
                nc.sync.dma_start(
                    out=xw[:, k, :],
                    in_=xT_d[w, k * 128:(k + 1) * 128, :],
                )
            xw_tiles[w] = xw

        prefetch_x(0)

        wq_sb = const.tile([128, FT, 3 * DIM], bf16)
        # qk columns first, in per-group chunks — the prologue's m-th matmul
        # group can start as soon as its own 128 columns have landed
        for m in range(2 * FT):
            nc.sync.dma_start(
                out=wq_sb[:, :, m * 128:(m + 1) * 128],
                in_=wq_d[:, m * 128:(m + 1) * 128].rearrange("(k p) c -> p k c", p=128),
            )
        nc.sync.dma_start(out=wq_sb[:, :, 2 * DIM:], in_=wq_d[:, 2 * DIM:].rearrange("(k p) c -> p k c", p=128))
        bqk_sb = const.tile([128, 2 * FT], f32)
        nc.sync.dma_start(out=bqk_sb, in_=bq_d[:, :])
        # full exp-bias table, SBUF-resident; chunked by head pair so the
        # first attention rounds don't wait for the whole 6.3MB
        expb_sb = const.tile([128, NH, KT, NTOK], bf16)
        for hp in range(NH // 2):
            nc.sync.dma_start(
                out=expb_sb[:, 2 * hp:2 * hp + 2, :, :],
                in_=eb_d[:, 2 * hp:2 * hp + 2, :, :],
            )
        wp_sb = const.tile([128, FT, DIM], bf16)
        nc.sync.dma_start(out=wp_sb, in_=wp_d[:, :].rearrange("(k p) c -> p k c", p=128))
        prefetch_x(1)

        qT = const.tile([128, 2, FT, NTOK], bf16)         # q transposed, 2 window slots
        kpad = const.tile([128, 2, NH, NTOK], bf16)       # k transposed, zero-padded per head
        # pad rows stay zero across windows; memzero runs on the (idle at
        # prologue) Scalar engine, keeping DVE free for the first copies
        nc.scalar.memzero(kpad)
        # v natural + 64 ones columns: the PV matmul then emits the softmax
        # denominator REPLICATED on PSUM partitions 64..127 (matmul cost
        # depends only on the moving free size, so the extra columns are
        # free) — no cross-partition broadcast needed for the normalize
        vsb = const.tile([128, 2, KT, NH, 2 * HD], bf16)
        nc.vector.memset(vsb[:, 0, :, :, :], 1.0)         # ones survive the v copies
        nc.vector.memset(vsb[:, 1, :, :, :], 1.0)
        aoT = const.tile([128, 2, FT, NTOK], bf16)        # attn output, transposed

        # ---- fill queue: single-instruction-granularity PE filler work ----
        from collections import deque

        fill_q = deque()
        fill_cur = [None]

        def fill(n):
            for _ in range(n):
                while True:
                    if fill_cur[0] is None:
                        if not fill_q:
                            return
                        fill_cur[0] = fill_q.popleft()()
                    try:
                        next(fill_cur[0])
                        break
                    except StopIteration:
                        fill_cur[0] = None

        def fill_all():
            fill(1 << 30)

        def qk_steps(s, xw, m):
            ps = ps_f.tile([128, 512], f32, name="psf", tag="psf")
            for k in range(FT):
                nc.tensor.matmul(
                    ps,
                    wq_sb[:, k, m * 128:(m + 1) * 128],
                    xw[:, k, :],
                    start=(k == 0), stop=(k == FT - 1),
                )
                yield
            if m < FT:
                # q half: packed head pair, straight copy
                if has_bqk:
                    nc.scalar.activation(
                        out=qT[:, s, m, :], in_=ps, func=AF.Identity,
                        bias=bqk_sb[:, m:m + 1], scale=1.0,
                    )
                else:
                    nc.vector.tensor_copy(out=qT[:, s, m, :], in_=ps)
            else:
                # k half: split the head pair into two zero-padded tiles;
                # the pad rows were zeroed once at start and never rewritten
                g = m - FT
                if has_bqk:
                    nc.scalar.activation(
                        out=kpad[0:64, s, 2 * g, :], in_=ps[0:64, :], func=AF.Identity,
                        bias=bqk_sb[0:64, m:m + 1], scale=1.0,
                    )
                    nc.scalar.activation(
                        out=kpad[64:128, s, 2 * g + 1, :], in_=ps[64:128, :], func=AF.Identity,
                        bias=bqk_sb[64:128, m:m + 1], scale=1.0,
                    )
                else:
                    nc.scalar.copy(out=kpad[0:64, s, 2 * g, :], in_=ps[0:64, :])
                    nc.vector.tensor_copy(out=kpad[64:128, s, 2 * g + 1, :], in_=ps[64:128, :])
            yield

        def v_steps(s, xw, mt, n):
            ps = ps_f.tile([128, 512], f32, name="psf", tag="psf")
            for k in range(FT):
                nc.tensor.matmul(
                    ps[:, 0:384],
                    xw[:, k, mt * 128:(mt + 1) * 128],
                    wq_sb[:, k, 2 * DIM + n * 384: 2 * DIM + (n + 1) * 384],
                    start=(k == 0), stop=(k == FT - 1),
                )
                yield
            nc.vector.tensor_copy(
                out=vsb[:, s, mt, n * 6:(n + 1) * 6, 0:HD],
                in_=ps[:, 0:384].rearrange("p (j c) -> p j c", c=HD),
            )
            yield

        def proj_steps(s, w, mt):
            osb = opool.tile([128, DIM], bf16, name="osb", tag="osb")
            for n in range(2):
                ps = ps_f.tile([128, 512], f32, name="psf", tag="psf")
                for k in range(FT):
                    nc.tensor.matmul(
                        ps[:, 0:384],
                        aoT[:, s, k, mt * 128:(mt + 1) * 128],
                        wp_sb[:, k, n * 384:(n + 1) * 384],
                        start=(k == 0), stop=(k == FT - 1),
                    )
                    yield
                nc.vector.tensor_copy(out=osb[:, n * 384:(n + 1) * 384], in_=ps[:, 0:384])
                yield
            nc.sync.dma_start(out=out_d[w, mt * 128:(mt + 1) * 128, :], in_=osb)
            yield

        def push_qkv_fills(w, skip_v1=False):
            s = w % 2
            xw = xw_tiles.pop(w)
            for m in range(2 * FT):
                fill_q.append(lambda s=s, xw=xw, m=m: qk_steps(s, xw, m))
            for n in range(0 if skip_v1 else 2):
                for mt in range(KT):
                    fill_q.append(lambda s=s, xw=xw, mt=mt, n=n: v_steps(s, xw, mt, n))
            return xw

        def push_v1(w, xw):
            s = w % 2
            for n in range(2):
                for mt in range(KT):
                    fill_q.append(lambda s=s, xw=xw, mt=mt, n=n: v_steps(s, xw, mt, n))

        def push_proj(w):
            s = w % 2
            for mt in range(KT):
                fill_q.append(lambda s=s, w=w, mt=mt: proj_steps(s, w, mt))

        # ---- attention -----------------------------------------------------
        def emit_pv_mms(st, pv_ps):
            h, s, em = st
            for kt in range(KT):
                nc.tensor.matmul(
                    pv_ps,
                    vsb[:, s, kt, h, :],
                    em[:, h % 2, kt, :],
                    start=(kt == 0), stop=(kt == KT - 1),
                )

        def emit_pv_epilogue(st, pv_ps):
            # deferred past the head pair's exps so the ScalarE copy never
            # delays an exp that releases a score PSUM pair-tile
            h, s, _ = st
            po = (h % 2) * 64
            mq = h // 2
            # one copy stages everything (S rows 0..63, attn-out 64..127)
            # and frees the PSUM bank; recip and multiply run off-bank, the
            # multiply on the otherwise-idle GpSimd engine
            # rows 64..127 of pv_ps all hold the sum row S
            s64 = rpool.tile([HD, 512], f32, name="s64", tag="s64")
            nc.scalar.copy(out=s64, in_=pv_ps[HD:2 * HD, :])
            r64 = rpool.tile([HD, 512], f32, name="r64", tag="r64")
            # reciprocal_approx_fast misreads PSUM sources and any nonzero
            # base partition — in and out must both sit at partition 0
            nc.vector.reciprocal_approx_fast(out=r64, in_=s64)
            nc.vector.tensor_mul(
                out=aoT[po:po + 64, s, mq, :], in0=pv_ps[0:HD, :], in1=r64,
            )

        def pv_pair_tile():
            # both PVs of a head pair share one 2-bank tile; it frees as a
            # unit once both normalize chains drain
            return ps_pv.tile([128, 1024], f32, name="pv", tag="pv")

        pending = []
        pvt = [None]

        def emit_attn(w, hp):
            nonlocal pending
            s = w % 2
            # both heads of the pair: even head in half 0, odd in half 1
            em = empool.tile([128, 2, KT, NTOK], bf16, name="em", tag="em")

            for kt in range(KT):
                ps = ps_s.tile([128, 1024], f32, name="pss", tag="pss")
                nc.tensor.matmul(
                    ps[:, 0:512],
                    kpad[:, s, 2 * hp, kt * 128:(kt + 1) * 128],
                    qT[:, s, hp, :],
                    start=True, stop=True,
                )
                nc.tensor.matmul(
                    ps[:, 512:1024],
                    kpad[:, s, 2 * hp + 1, kt * 128:(kt + 1) * 128],
                    qT[:, s, hp, :],
                    start=True, stop=True,
                )
                # one exp covers both heads' score tiles (2 PSUM banks)
                nc.scalar.activation(out=em[:, :, kt, :], in_=ps, func=AF.Exp)
                # extra fill after the last pair: the next head pair's first
                # score stalls on this pair's exp releasing its PSUM tile
                fill(7 if kt == KT - 1 else 5)
                # PVs of the previous pair go late in this pair so their em
                # bias-multiplies (end of previous emit_attn) have drained
                if kt == 2 and pending:
                    pvt[0] = pv_pair_tile()
                    emit_pv_mms(pending[0], pvt[0][:, 0:512])
                    emit_pv_epilogue(pending[0], pvt[0][:, 0:512])
                    fill(3)
                if kt == 3 and pending:
                    emit_pv_mms(pending[1], pvt[0][:, 512:1024])
                    emit_pv_epilogue(pending[1], pvt[0][:, 512:1024])
                    fill(3)
            nc.vector.tensor_mul(
                out=em[:, 0, :, :], in0=em[:, 0, :, :], in1=expb_sb[:, 2 * hp, :, :],
            )
            nc.vector.tensor_mul(
                out=em[:, 1, :, :], in0=em[:, 1, :, :], in1=expb_sb[:, 2 * hp + 1, :, :],
            )
            pending = [(2 * hp, s, em), (2 * hp + 1, s, em)]

        # prologue: window 0's qkv fully emitted before its attention reads
        # it (Tile links deps by emission order)
        push_qkv_fills(0)
        fill_all()
        xw3 = None
        for w in range(BW):
            if w + 2 < BW:
                prefetch_x(w + 2)
            if w + 1 < BW:
                # defer window 3's second v half into window 3 so its lean
                # (no-qkv) stretch keeps some PE fill work
                xw3 = push_qkv_fills(w + 1, skip_v1=(w + 1 == BW - 1))
            elif xw3 is not None:
                push_v1(BW - 1, xw3)
                xw3 = None
            for hp in range(NH // 2):
                emit_attn(w, hp)
                if hp == 0 and w > 0:
                    push_proj(w - 1)
            # all of window w+1's qkv (and stray proj) must be emitted before
            # window w+1's attention reads it
            fill_all()
        pvt[0] = pv_pair_tile()
        emit_pv_mms(pending[0], pvt[0][:, 0:512])
        emit_pv_epilogue(pending[0], pvt[0][:, 0:512])
        emit_pv_mms(pending[1], pvt[0][:, 512:1024])
        emit_pv_epilogue(pending[1], pvt[0][:, 512:1024])
        pending = []
        push_proj(BW - 1)
        fill_all()

    if not nc.is_finalized():
        nc.finalize()
    return nc


def _host_prep(x, Wqkv, bqkv, rel_pos_bias_table, rel_pos_index):
    bf16 = ml_dtypes.bfloat16
    x = np.asarray(x, np.float32)
    Wqkv = np.asarray(Wqkv, np.float32)
    bqkv = np.asarray(bqkv, np.float32)
    table = np.asarray(rel_pos_bias_table, np.float32)
    idx = np.asarray(rel_pos_index)

    wqkvT = Wqkv.T.copy()               # [768, 2304]
    wqkvT[:, :DIM] *= SCALE             # fold softmax scale into q weights
    wqkvT_bf = wqkvT.astype(bf16)

    bqk = bqkv[:2 * DIM].copy()
    bqk[:DIM] *= SCALE
    has_bqk = bool(np.any(bqk))
    bqk_packed = np.ascontiguousarray(bqk.reshape(2 * FT, 128).T, dtype=np.float32)

    # expb[p, h, kt, q] = exp(bias_h[q, k]) with k = kt*128+p  (scores are transposed)
    E = np.exp(table[idx])              # [q, k, h]
    eb = E.transpose(1, 2, 0)           # [k, h, q]
    eb = eb.reshape(KT, 128, NH, NTOK).transpose(1, 2, 0, 3)   # [p, h, kt, q]
    eb_bf = np.ascontiguousarray(eb, dtype=bf16)

    xT = x.reshape(NCORES, BW, NTOK, DIM).transpose(0, 1, 3, 2)  # [core, w, feat, tok]
    xT_bf = np.ascontiguousarray(xT, dtype=bf16)
    return xT_bf, wqkvT_bf, bqk_packed, has_bqk, eb_bf


def kernel(x, Wqkv, bqkv, rel_pos_bias_table, rel_pos_index, Wproj, bproj):
    global LAST_RESULT
    from concourse.bass_utils import run_bass_kernel_spmd

    Wproj = np.asarray(Wproj, np.float32)
    bproj = np.asarray(bproj, np.float32)
    bqkv_np = np.asarray(bqkv, np.float32)

    xT_bf, wqkvT_bf, bqk_packed, has_bqk, eb_bf = _host_prep(
        x, Wqkv, bqkv_np, rel_pos_bias_table, rel_pos_index
    )
    wprojT_bf = np.ascontiguousarray(Wproj.T, dtype=ml_dtypes.bfloat16)

    key = has_bqk
    if key not in _nc_cache:
        _nc_cache[key] = _build(has_bqk)
    nc = _nc_cache[key]

    in_maps = [
        {
            "xT": xT_bf[c],
            "wqkvT": wqkvT_bf,
            "wprojT": wprojT_bf,
            "expb": eb_bf,
            "bqk": bqk_packed,
        }
        for c in range(NCORES)
    ]
    res = run_bass_kernel_spmd(
        nc, in_maps, list(range(NCORES)),
        trace=TRACE, trace_cores=[0] if TRACE else None,
    )
    LAST_RESULT = res
    out = np.concatenate(
        [np.asarray(res.results[c]["out"], dtype=np.float32) for c in range(NCORES)],
        axis=0,
    )

    # v-bias and proj-bias enter the output linearly; apply exactly on host.
    corr = bproj + bqkv_np[2 * DIM:] @ Wproj.T
    if np.any(corr):
        out = out + corr.astype(np.float32)
    return np.ascontiguousarray(out, dtype=np.float32)


# revision 104
# speedup vs baseline: 1.0180x; 1.0180x over previous
"""Trainium2 (8 NeuronCores, SPMD) kernel for windowed multi-head attention
with relative position bias (Swin-3D style block).

Strategy: pure data-parallel over the B=32 window axis — 4 windows per core,
no collectives. Per core:
  phase 1: qkv projection.  q produced TRANSPOSED (feature-on-partition) for
           the score matmuls; k produced transposed AND zero-padded to the
           full 128-partition contraction (head h's 64 features live in
           rows (h%2)*64..+64, the other 64 rows stay zero) so score
           matmuls keep the PE in its 128-row tile config — mixing 64-row
           and 128-row stationaries costs a ~100ns array-reconfig on every
           transition.  v produced natural (token-on-partition) with a
           column of ones appended (row 64 of the PV output then holds the
           softmax denominator for free).
  phase 2: per (head pair, window): both heads' scores^T land in one
           2-bank PSUM tile, one exp on ScalarE covers both, then multiply
           by exp(bias)^T (host-precomputed, fully SBUF-resident), PV
           matmuls accumulate attn_out^T [64+1, 512], normalize by the
           reciprocal of the sum row (partition-broadcast on the otherwise
           idle GpSimd engine).
  phase 3: output projection from attn_out^T tiles; result lands natural
           [token, feature] and is DMAed out.

All matmul operands are bf16 (full TensorE rate); accumulation fp32 in PSUM.
The softmax scale is folded into the q weights on the host. exp(s+b) is
computed as exp(s)*exp(b) — exact up to fp rounding, and lets the bias add
run as a cheap bf16 multiply on VectorE instead of an fp32 PSUM add.
v/proj biases enter the output linearly and are applied on the host (they
are zeros for this problem's inputs anyway).

Scheduling: the per-engine instruction streams are STATIC after Tile
scheduling.  qkv matmuls for the NEXT window and projection matmuls for the
PREVIOUS window are interleaved between score/PV instructions at
single-matmul granularity (generator-based fill queue) so the PE never
idles while ScalarE drains exp chains.  qkT/kpad/vsb/aoT are 2-slot
rotating buffers (window parity) so the full exp-bias table fits in SBUF;
x tiles are prefetched 2 windows ahead.
"""

import os

import numpy as np
import ml_dtypes

# the device can be left in a sticky slow mode (~+55us on every launch,
# persisting across runs) by prior sessions; a core reset at open clears it
os.environ.setdefault("NEURON_RT_RESET_CORES", "1")

B, NTOK, DIM = 32, 512, 768
NH, HD = 12, 64
NCORES = 8
BW = B // NCORES          # 4 windows per core
SCALE = HD ** -0.5
KT = NTOK // 128          # 4 token tiles
FT = DIM // 128           # 6 feature tiles

TRACE = False             # set by test.py to capture neuron-profile timing
LAST_RESULT = None        # BassKernelResults of the last run (for profiling)

_nc_cache = {}


def _build(has_bqk: bool):
    import concourse.mybir as mybir
    import concourse.tile as tile
    from concourse import bacc
    from contextlib import ExitStack

    dt = mybir.dt
    bf16, f32 = dt.bfloat16, dt.float32
    AF = mybir.ActivationFunctionType

    # Bacc (not plain Bass): its compile pass splits multi-semaphore waits
    # into EventSemaphore instructions — TRN2 allows only 1 wait per inst.
    nc = bacc.Bacc("TRN2", target_bir_lowering=False, debug=False)
    xT_d = nc.declare_dram_parameter("xT", [BW, DIM, NTOK], bf16, False)
    wq_d = nc.declare_dram_parameter("wqkvT", [DIM, 3 * DIM], bf16, False)
    wp_d = nc.declare_dram_parameter("wprojT", [DIM, DIM], bf16, False)
    eb_d = nc.declare_dram_parameter("expb", [128, NH, KT, NTOK], bf16, False)
    bq_d = nc.declare_dram_parameter("bqk", [128, 2 * FT], f32, False)
    out_d = nc.declare_dram_parameter("out", [BW, NTOK, DIM], bf16, True)

    ctx = ExitStack()
    with ctx:
        tc = ctx.enter_context(tile.TileContext(nc))
        const = ctx.enter_context(tc.tile_pool(name="const", bufs=1))
        xpool = ctx.enter_context(tc.tile_pool(name="xT", bufs=2))
        empool = ctx.enter_context(tc.tile_pool(name="expm", bufs=3))
        rpool = ctx.enter_context(tc.tile_pool(name="recip", bufs=3))
        opool = ctx.enter_context(tc.tile_pool(name="osb", bufs=2))
        ps_s = ctx.enter_context(tc.tile_pool(name="ps_s", bufs=2, space="PSUM"))
        ps_f = ctx.enter_context(tc.tile_pool(name="ps_f", bufs=2, space="PSUM"))
        ps_pv = ctx.enter_context(tc.tile_pool(name="ps_pv", bufs=1, space="PSUM"))

        xw_tiles = {}

        def prefetch_x(w):
            xw = xpool.tile([128, FT, NTOK], bf16, name="xw", tag="xw")
            # per-k chunks: the first qkv matmuls need only the k=0 slice
            for k in range(FT):
                nc.sync.dma_start(
                    out=xw[:, k, :],
                    in_=xT_d[w, k * 128:(k + 1) * 128, :],
                )
            xw_tiles[w] = xw

        prefetch_x(0)

        wq_sb = const.tile([128, FT, 3 * DIM], bf16)
        # qk columns first, in per-group chunks — the prologue's m-th matmul
        # group can start as soon as its own 128 columns have landed
        for m in range(2 * FT):
            nc.sync.dma_start(
                out=wq_sb[:, :, m * 128:(m + 1) * 128],
                in_=wq_d[:, m * 128:(m + 1) * 128].rearrange("(k p) c -> p k c", p=128),
            )
        nc.sync.dma_start(out=wq_sb[:, :, 2 * DIM:], in_=wq_d[:, 2 * DIM:].rearrange("(k p) c -> p k c", p=128))
        bqk_sb = const.tile([128, 2 * FT], f32)
        nc.sync.dma_start(out=bqk_sb, in_=bq_d[:, :])
        # full exp-bias table, SBUF-resident; chunked by head pair so the
        # first attention rounds don't wait for the whole 6.3MB
        expb_sb = const.tile([128, NH, KT, NTOK], bf16)
        for hp in range(NH // 2):
            nc.sync.dma_start(
                out=expb_sb[:, 2 * hp:2 * hp + 2, :, :],
                in_=eb_d[:, 2 * hp:2 * hp + 2, :, :],
            )
        wp_sb = const.tile([128, FT, DIM], bf16)
        nc.sync.dma_start(out=wp_sb, in_=wp_d[:, :].rearrange("(k p) c -> p k c", p=128))
        prefetch_x(1)

        qT = const.tile([128, 2, FT, NTOK], bf16)         # q transposed, 2 window slots
        kpad = const.tile([128, 2, NH, NTOK], bf16)       # k transposed, zero-padded per head
        # pad rows stay zero across windows; memzero runs on the (idle at
        # prologue) Scalar engine, keeping DVE free for the first copies
        nc.scalar.memzero(kpad)
        # v natural + 64 ones columns: the PV matmul then emits the softmax
        # denominator REPLICATED on PSUM partitions 64..127 (matmul cost
        # depends only on the moving free size, so the extra columns are
        # free) — no cross-partition broadcast needed for the normalize
        vsb = const.tile([128, 2, KT, NH, 2 * HD], bf16)
        nc.vector.memset(vsb[:, 0, :, :, :], 1.0)         # ones survive the v copies
        nc.vector.memset(vsb[:, 1, :, :, :], 1.0)
        aoT = const.tile([128, 2, FT, NTOK], bf16)        # attn output, transposed

        # ---- fill queue: single-instruction-granularity PE filler work ----
        from collections import deque

        fill_q = deque()
        fill_cur = [None]

        def fill(n):
            for _ in range(n):
                while True:
                    if fill_cur[0] is None:
                        if not fill_q:
                            return
                        fill_cur[0] = fill_q.popleft()()
                    try:
                        next(fill_cur[0])
                        break
                    except StopIteration:
                        fill_cur[0] = None

        def fill_all():
            fill(1 << 30)

        def qk_steps(s, xw, m):
            ps = ps_f.tile([128, 512], f32, name="psf", tag="psf")
            for k in range(FT):
                nc.tensor.matmul(
                    ps,
                    wq_sb[:, k, m * 128:(m + 1) * 128],
                    xw[:, k, :],
                    start=(k == 0), stop=(k == FT - 1),
                )
                yield
            if m < FT:
                # q half: packed head pair, straight copy
                if has_bqk:
                    nc.scalar.activation(
                        out=qT[:, s, m, :], in_=ps, func=AF.Identity,
                        bias=bqk_sb[:, m:m + 1], scale=1.0,
                    )
                else:
                    nc.vector.tensor_copy(out=qT[:, s, m, :], in_=ps)
            else:
                # k half: split the head pair into two zero-padded tiles;
                # the pad rows were zeroed once at start and never rewritten
                g = m - FT
                if has_bqk:
                    nc.scalar.activation(
                        out=kpad[0:64, s, 2 * g, :], in_=ps[0:64, :], func=AF.Identity,
                        bias=bqk_sb[0:64, m:m + 1], scale=1.0,
                    )
                    nc.scalar.activation(
                        out=kpad[64:128, s, 2 * g + 1, :], in_=ps[64:128, :], func=AF.Identity,
                        bias=bqk_sb[64:128, m:m + 1], scale=1.0,
                    )
                else:
                    nc.scalar.copy(out=kpad[0:64, s, 2 * g, :], in_=ps[0:64, :])
                    nc.vector.tensor_copy(out=kpad[64:128, s, 2 * g + 1, :], in_=ps[64:128, :])
            yield

        def v_steps(s, xw, mt, n):
            ps = ps_f.tile([128, 512], f32, name="psf", tag="psf")
            for k in range(FT):
                nc.tensor.matmul(
                    ps[:, 0:384],
                    xw[:, k, mt * 128:(mt + 1) * 128],
                    wq_sb[:, k, 2 * DIM + n * 384: 2 * DIM + (n + 1) * 384],
                    start=(k == 0), stop=(k == FT - 1),
                )
                yield
            nc.vector.tensor_copy(
                out=vsb[:, s, mt, n * 6:(n + 1) * 6, 0:HD],
                in_=ps[:, 0:384].rearrange("p (j c) -> p j c", c=HD),
            )
            yield

        def proj_steps(s, w, mt):
            osb = opool.tile([128, DIM], bf16, name="osb", tag="osb")
            for n in range(2):
                ps = ps_f.tile([128, 512], f32, name="psf", tag="psf")
                for k in range(FT):
                    nc.tensor.matmul(
                        ps[:, 0:384],
                        aoT[:, s, k, mt * 128:(mt + 1) * 128],
                        wp_sb[:, k, n * 384:(n + 1) * 384],
                        start=(k == 0), stop=(k == FT - 1),
                    )
                    yield
                nc.vector.tensor_copy(out=osb[:, n * 384:(n + 1) * 384], in_=ps[:, 0:384])
                yield
            nc.sync.dma_start(out=out_d[w, mt * 128:(mt + 1) * 128, :], in_=osb)
            yield

        def push_qkv_fills(w, skip_v1=False):
            s = w % 2
            xw = xw_tiles.pop(w)
            for m in range(2 * FT):
                fill_q.append(lambda s=s, xw=xw, m=m: qk_steps(s, xw, m))
            for n in range(0 if skip_v1 else 2):
                for mt in range(KT):
                    fill_q.append(lambda s=s, xw=xw, mt=mt, n=n: v_steps(s, xw, mt, n))
            return xw

        def push_v1(w, xw):
            s = w % 2
            for n in range(2):
                for mt in range(KT):
                    fill_q.append(lambda s=s, xw=xw, mt=mt, n=n: v_steps(s, xw, mt, n))

        def push_proj(w):
            s = w % 2
            for mt in range(KT):
                fill_q.append(lambda s=s, w=w, mt=mt: proj_steps(s, w, mt))

        # ---- attention -----------------------------------------------------
        def emit_pv_mms(st, pv_ps):
            h, s, em = st
            for kt in range(KT):
                nc.tensor.matmul(
                    pv_ps,
                    vsb[:, s, kt, h, :],
                    em[:, h % 2, kt, :],
                    start=(kt == 0), stop=(kt == KT - 1),
                )

        def emit_pv_epilogue(st, pv_ps):
            # deferred past the head pair's exps so the ScalarE copy never
            # delays an exp that releases a score PSUM pair-tile
            h, s, _ = st
            po = (h % 2) * 64
            mq = h // 2
            # one copy stages everything (S rows 0..63, attn-out 64..127)
            # and frees the PSUM bank; recip and multiply run off-bank, the
            # multiply on the otherwise-idle GpSimd engine
            # rows 64..127 of pv_ps all hold the sum row S
            s64 = rpool.tile([HD, 512], f32, name="s64", tag="s64")
            nc.scalar.copy(out=s64, in_=pv_ps[HD:2 * HD, :])
            r64 = rpool.tile([HD, 512], f32, name="r64", tag="r64")
            # reciprocal_approx_fast misreads PSUM sources and any nonzero
            # base partition — in and out must both sit at partition 0
            nc.vector.reciprocal_approx_fast(out=r64, in_=s64)
            nc.vector.tensor_mul(
                out=aoT[po:po + 64, s, mq, :], in0=pv_ps[0:HD, :], in1=r64,
            )

        def pv_pair_tile():
            # both PVs of a head pair share one 2-bank tile; it frees as a
            # unit once both normalize chains drain
            return ps_pv.tile([128, 1024], f32, name="pv", tag="pv")

        pending = []
        pvt = [None]

        def emit_attn(w, hp):
            nonlocal pending
            s = w % 2
            # both heads of the pair: even head in half 0, odd in half 1
            em = empool.tile([128, 2, KT, NTOK], bf16, name="em", tag="em")

            for kt in range(KT):
                ps = ps_s.tile([128, 1024], f32, name="pss", tag="pss")
                nc.tensor.matmul(
                    ps[:, 0:512],
                    kpad[:, s, 2 * hp, kt * 128:(kt + 1) * 128],
                    qT[:, s, hp, :],
                    start=True, stop=True,
                )
                nc.tensor.matmul(
                    ps[:, 512:1024],
                    kpad[:, s, 2 * hp + 1, kt * 128:(kt + 1) * 128],
                    qT[:, s, hp, :],
                    start=True, stop=True,
                )
                # one exp covers both heads' score tiles (2 PSUM banks)
                nc.scalar.activation(out=em[:, :, kt, :], in_=ps, func=AF.Exp)
                # extra fill after the last pair: the next head pair's first
                # score stalls on this pair's exp releasing its PSUM tile
                fill(7 if kt == KT - 1 else 5)
                # PVs of the previous pair go late in this pair so their em
                # bias-multiplies (end of previous emit_attn) have drained
                if kt == 2 and pending:
                    pvt[0] = pv_pair_tile()
                    emit_pv_mms(pending[0], pvt[0][:, 0:512])
                    emit_pv_epilogue(pending[0], pvt[0][:, 0:512])
                    fill(2)
                if kt == 3 and pending:
                    emit_pv_mms(pending[1], pvt[0][:, 512:1024])
                    emit_pv_epilogue(pending[1], pvt[0][:, 512:1024])
                    fill(2)
            nc.vector.tensor_mul(
                out=em[:, 0, :, :], in0=em[:, 0, :, :], in1=expb_sb[:, 2 * hp, :, :],
            )
            nc.vector.tensor_mul(
                out=em[:, 1, :, :], in0=em[:, 1, :, :], in1=expb_sb[:, 2 * hp + 1, :, :],
            )
            pending = [(2 * hp, s, em), (2 * hp + 1, s, em)]

        # prologue: window 0's qkv fully emitted before its attention reads
        # it (Tile links deps by emission order)
        push_qkv_fills(0)
        fill_all()
        xw3 = None
        for w in range(BW):
            if w + 2 < BW:
                prefetch_x(w + 2)
            if w + 1 < BW:
                # defer window 3's second v half into window 3 so its lean
                # (no-qkv) stretch keeps some PE fill work
                xw3 = push_qkv_fills(w + 1, skip_v1=(w + 1 == BW - 1))
            elif xw3 is not None:
                push_v1(BW - 1, xw3)
                xw3 = None
            for hp in range(NH // 2):
                emit_attn(w, hp)
                if hp == 0 and w > 0:
                    push_proj(w - 1)
            # all of window w+1's qkv (and stray proj) must be emitted before
            # window w+1's attention reads it
            fill_all()
        pvt[0] = pv_pair_tile()
        emit_pv_mms(pending[0], pvt[0][:, 0:512])
        emit_pv_epilogue(pending[0], pvt[0][:, 0:512])
        emit_pv_mms(pending[1], pvt[0][:, 512:1024])
        emit_pv_epilogue(pending[1], pvt[0][:, 512:1024])
        pending = []
        push_proj(BW - 1)
        fill_all()

    if not nc.is_finalized():
        nc.finalize()
    return nc


def _host_prep(x, Wqkv, bqkv, rel_pos_bias_table, rel_pos_index):
    bf16 = ml_dtypes.bfloat16
    x = np.asarray(x, np.float32)
    Wqkv = np.asarray(Wqkv, np.float32)
    bqkv = np.asarray(bqkv, np.float32)
    table = np.asarray(rel_pos_bias_table, np.float32)
    idx = np.asarray(rel_pos_index)

    wqkvT = Wqkv.T.copy()               # [768, 2304]
    wqkvT[:, :DIM] *= SCALE             # fold softmax scale into q weights
    wqkvT_bf = wqkvT.astype(bf16)

    bqk = bqkv[:2 * DIM].copy()
    bqk[:DIM] *= SCALE
    has_bqk = bool(np.any(bqk))
    bqk_packed = np.ascontiguousarray(bqk.reshape(2 * FT, 128).T, dtype=np.float32)

    # expb[p, h, kt, q] = exp(bias_h[q, k]) with k = kt*128+p  (scores are transposed)
    E = np.exp(table[idx])              # [q, k, h]
    eb = E.transpose(1, 2, 0)           # [k, h, q]
    eb = eb.reshape(KT, 128, NH, NTOK).transpose(1, 2, 0, 3)   # [p, h, kt, q]
    eb_bf = np.ascontiguousarray(eb, dtype=bf16)

    xT = x.reshape(NCORES, BW, NTOK, DIM).transpose(0, 1, 3, 2)  # [core, w, feat, tok]
    xT_bf = np.ascontiguousarray(xT, dtype=bf16)
    return xT_bf, wqkvT_bf, bqk_packed, has_bqk, eb_bf


def kernel(x, Wqkv, bqkv, rel_pos_bias_table, rel_pos_index, Wproj, bproj):
    global LAST_RESULT
    from concourse.bass_utils import run_bass_kernel_spmd

    Wproj = np.asarray(Wproj, np.float32)
    bproj = np.asarray(bproj, np.float32)
    bqkv_np = np.asarray(bqkv, np.float32)

    xT_bf, wqkvT_bf, bqk_packed, has_bqk, eb_bf = _host_prep(
        x, Wqkv, bqkv_np, rel_pos_bias_table, rel_pos_index
    )
    wprojT_bf = np.ascontiguousarray(Wproj.T, dtype=ml_dtypes.bfloat16)

    key = has_bqk
    if key not in _nc_cache:
        _nc_cache[key] = _build(has_bqk)
    nc = _nc_cache[key]

    in_maps = [
        {
            "xT": xT_bf[c],
            "wqkvT": wqkvT_bf,
            "wprojT": wprojT_bf,
            "expb": eb_bf,
            "bqk": bqk_packed,
        }
        for c in range(NCORES)
    ]
    res = run_bass_kernel_spmd(
        nc, in_maps, list(range(NCORES)),
        trace=TRACE, trace_cores=[0] if TRACE else None,
    )
    LAST_RESULT = res
    out = np.concatenate(
        [np.asarray(res.results[c]["out"], dtype=np.float32) for c in range(NCORES)],
        axis=0,
    )

    # v-bias and proj-bias enter the output linearly; apply exactly on host.
    corr = bproj + bqkv_np[2 * DIM:] @ Wproj.T
    if np.any(corr):
        out = out + corr.astype(np.float32)
    return np.ascontiguousarray(out, dtype=np.float32)
